# revision 5
# baseline (speedup 1.0000x reference)
"""Trainium2 Bass kernel for nn_BaseModel_63058709840114 (dense_mlp), v14 = v13 + relu parity-split across DVE/ACT (DVE was the 93%-busy bottleneck; ACT has wait-free slack after the stage split).

Folded math (host, fp64):
    hc  = x @ A1 + c1            (A1 = We@C centered, c1 = be@C)
    r1  = rsqrt(mean(hc^2) + eps)
    a   = relu((hc*r1) @ W1g + b1f)
    h2c = hc + a @ W2C + c12 - c1
    r2  = rsqrt(mean(h2c^2) + eps)
    out = (h2c*r2) @ Whg + bhf

v6 structure: a 10-stage software-pipelined wavefront at supertile
granularity ([120,512] tiles, 5120 rows each). Iteration T emits stage S_i
for supertile T-i, so every instruction's inputs were produced >= 1
iteration earlier and all engines stream without intra-chain stalls:

  S0 ep-mm + hcs evac (DVE ts +c1)       S5 mp-mms(w2c + a1 resid) + h2c (ACT)
  S1 sq1 = hcs^2 (POOL)                  S6 sq2 = h2c^2 (POOL)
  S2 v1-mm + r1 rsqrt (ACT)              S7 v2-mm + r2 rsqrt (ACT)
  S3 n1 = hcs*r1 (DVE)                   S8 n2 = h2c*r2 (DVE)
  S4 z-mm + relu (DVE ts)                S9 head-mms + out (DVE tt) + DMA

PSUM singles: ep(2) v1(2) zp(2) mp(1) hp(1) = 8 banks.
x is host-pretransposed to blocked feature-major bf16 [n_quad, 100, 2048];
rows map as row = (s*128 + b)*40 + c*10 + t.
"""

import os
import sys
import numpy as np
import ml_dtypes

sys.path.insert(0, "/opt/trn_rl_repo")

EPS = 1e-5
D_IN, D = 10, 12
G = 10
NCOL = 512
ROWS_ST = 128 * G * 4       # 5120
N_CORES = 8
B_FULL = 4194304


def _fold_weights(w):
    f64 = {k: np.asarray(v, dtype=np.float64) for k, v in w.items()}
    C = np.eye(D) - np.ones((D, D)) / D
    A1 = f64["w_embed"] @ C
    c1 = f64["b_embed"] @ C
    W1g = np.diag(f64["g_norm1"]) @ f64["w_fc1"]
    b1f = f64["b_norm1"] @ f64["w_fc1"] + f64["b_fc1"]
    W2C = f64["w_fc2"] @ C
    c2 = f64["b_fc2"] @ C
    Whg = np.diag(f64["g_normh"]) @ f64["w_head"]
    bhf = f64["b_normh"] @ f64["w_head"] + f64["b_head"]
    return dict(A1=A1, c1=c1, W1g=W1g, b1f=b1f, W2C=W2C, c12=c1 + c2, Whg=Whg, bhf=bhf)


def _block_diag(M, nblk):
    k, m = M.shape
    out = np.zeros((nblk * k, nblk * m), dtype=M.dtype)
    for t in range(nblk):
        out[t * k:(t + 1) * k, t * m:(t + 1) * m] = M
    return out


def make_consts(w):
    f = _fold_weights(w)
    bf16 = ml_dtypes.bfloat16
    consts = {}
    consts["a1blk"] = _block_diag(f["A1"].astype(np.float32), G).astype(bf16)
    consts["w1gblk"] = _block_diag(f["W1g"].astype(np.float32), G).astype(bf16)
    consts["w2cblk"] = _block_diag(f["W2C"].astype(np.float32), G).astype(bf16)
    consts["whgblk"] = _block_diag(f["Whg"].astype(np.float32), G).astype(bf16)
    consts["vrep"] = _block_diag(np.ones((D, D), np.float32), G).astype(bf16)
    consts["c1v"] = np.tile(f["c1"], G).astype(np.float32).reshape(120, 1)
    consts["b1v"] = np.tile(f["b1f"], G).astype(np.float32).reshape(120, 1)
    consts["c12v"] = np.tile(f["c12"], G).astype(np.float32).reshape(120, 1)
    consts["bhnat"] = np.tile(f["bhf"].astype(np.float32), 40).reshape(1, 480).repeat(128, 0).copy()
    consts["epsv"] = np.full((128, 1), EPS, dtype=np.float32)
    return consts


CONST_SPECS = [
    ("a1blk", (100, 120), "bf16"),
    ("w1gblk", (120, 120), "bf16"),
    ("w2cblk", (120, 120), "bf16"),
    ("whgblk", (120, 120), "bf16"),
    ("vrep", (120, 120), "bf16"),
    ("c1v", (120, 1), "f32"),
    ("b1v", (120, 1), "f32"),
    ("c12v", (120, 1), "f32"),
    ("bhnat", (128, 480), "f32"),
    ("epsv", (128, 1), "f32"),
]


def build_nc(b_core):
    import concourse.bacc as bacc
    import concourse.mybir as mybir
    import concourse.tile as tile

    dt = mybir.dt
    BF, F32 = dt.bfloat16, dt.float32
    AF = mybir.ActivationFunctionType
    OP = mybir.AluOpType

    assert b_core % (ROWS_ST * 4) == 0
    n_st = b_core // ROWS_ST
    n_quad = n_st // 4

    nc = bacc.Bacc("TRN2", target_bir_lowering=False, debug=False)
    x_d = nc.dram_tensor("x", [n_quad, 100, 4 * NCOL], BF, kind="ExternalInput")
    out_d = nc.dram_tensor("out", [b_core, D], F32, kind="ExternalOutput")
    cd = {}
    for name, shape, ty in CONST_SPECS:
        cd[name] = nc.dram_tensor(name, list(shape), BF if ty == "bf16" else F32,
                                  kind="ExternalInput")

    ov = out_d.ap().rearrange("(s b r) j -> s b (r j)", s=n_st, b=128, r=40)

    with tile.TileContext(nc) as tc:
        with (
            tc.tile_pool(name="const", bufs=1) as constp,
            tc.tile_pool(name="xin", bufs=4) as xinp,
            tc.tile_pool(name="ep", bufs=2, space="PSUM") as epp,
            tc.tile_pool(name="v1", bufs=3, space="PSUM") as v1p,
            tc.tile_pool(name="zp", bufs=1, space="PSUM") as zpp,
            tc.tile_pool(name="mp", bufs=1, space="PSUM") as mpp,
            tc.tile_pool(name="hp", bufs=1, space="PSUM") as hpp,
            tc.tile_pool(name="hcs", bufs=8) as hcsp,
            tc.tile_pool(name="sq1", bufs=5) as sq1p,
            tc.tile_pool(name="r1", bufs=4) as r1p,
            tc.tile_pool(name="n1", bufs=4) as n1p,
            tc.tile_pool(name="a", bufs=4) as ap_,
            tc.tile_pool(name="h2c", bufs=7) as h2cp,
            tc.tile_pool(name="sq2", bufs=5) as sq2p,
            tc.tile_pool(name="r2", bufs=4) as r2p,
            tc.tile_pool(name="n2", bufs=4) as n2p,
            tc.tile_pool(name="outf", bufs=4) as outfp,
        ):
            cs = {}
            for name, shape, ty in CONST_SPECS:
                t = constp.tile(list(shape), BF if ty == "bf16" else F32, tag=name)
                nc.sync.dma_start(out=t[:], in_=cd[name].ap())
                cs[name] = t

            st = {}   # supertile idx -> dict of tiles

            def load_quad(g):
                if g >= n_quad:
                    return
                xq = xinp.tile([100, 4 * NCOL], BF, tag="xq")
                nc.sync.dma_start(out=xq[:], in_=x_d.ap()[g])
                for i in range(4):
                    st[4 * g + i] = {"x": xq[:, NCOL * i:NCOL * (i + 1)]}

            def S0(s):
                if s % 4 == 0:
                    load_quad(s // 4 + 2)
                d = st[s]
                ep = epp.tile([120, NCOL], F32, tag="ep")
                nc.tensor.matmul(ep[:], cs["a1blk"][:], d["x"], start=True, stop=True)
                hcs = hcsp.tile([120, NCOL], BF, tag="hcs")
                nc.vector.tensor_scalar(hcs[:], ep[:], cs["c1v"][:, 0:1], None, OP.add)
                d["hcs"] = hcs

            def S1(s):
                d = st[s]
                sq1 = sq1p.tile([120, NCOL], BF, tag="sq1")
                nc.gpsimd.tensor_mul(sq1[:], d["hcs"][:], d["hcs"][:])
                d["sq1"] = sq1

            def S2a(s):
                d = st[s]
                v1 = v1p.tile([120, NCOL], F32, tag="v1")
                nc.tensor.matmul(v1[:], cs["vrep"][:], d.pop("sq1")[:],
                                 start=True, stop=True)
                d["v1"] = v1

            def S2b(s):
                d = st[s]
                r1 = r1p.tile([120, NCOL], BF, tag="r1")
                nc.scalar.activation(r1[:], d.pop("v1")[:], AF.Abs_reciprocal_sqrt,
                                     bias=cs["epsv"][0:120, 0:1], scale=1.0 / D)
                d["r1"] = r1

            def S3(s):
                d = st[s]
                n1 = n1p.tile([120, NCOL], BF, tag="n1")
                nc.vector.tensor_mul(n1[:], d["hcs"][:], d.pop("r1")[:])
                d["n1"] = n1

            def S4(s):
                d = st[s]
                zp = zpp.tile([120, NCOL], F32, tag="zp")
                nc.tensor.matmul(zp[:], cs["w1gblk"][:], d.pop("n1")[:],
                                 start=True, stop=True)
                a_s = ap_.tile([120, NCOL], BF, tag="a")
                if s % 2 == 0:
                    nc.vector.tensor_scalar(
                        a_s[:], zp[:], cs["b1v"][:, 0:1], 0.0, OP.add, OP.max)
                else:
                    nc.scalar.activation(a_s[:], zp[:], AF.Relu,
                                         bias=cs["b1v"][:, 0:1])
                d["a"] = a_s

            def S5(s):
                d = st[s]
                mp = mpp.tile([120, NCOL], F32, tag="mp")
                nc.tensor.matmul(mp[:], cs["w2cblk"][:], d.pop("a")[:],
                                 start=True, stop=False, skip_group_check=True)
                nc.tensor.matmul(mp[:], cs["a1blk"][:], d.pop("x"),
                                 start=False, stop=True, skip_group_check=True)
                h2c = h2cp.tile([120, NCOL], BF, tag="h2c")
                nc.scalar.activation(h2c[:], mp[:], AF.Identity,
                                     bias=cs["c12v"][:, 0:1])
                d.pop("hcs")
                d["h2c"] = h2c

            def S6(s):
                d = st[s]
                sq2 = sq2p.tile([120, NCOL], BF, tag="sq2")
                nc.gpsimd.tensor_mul(sq2[:], d["h2c"][:], d["h2c"][:])
                d["sq2"] = sq2

            def S7a(s):
                d = st[s]
                v2 = v1p.tile([120, NCOL], F32, tag="v1")
                nc.tensor.matmul(v2[:], cs["vrep"][:], d.pop("sq2")[:],
                                 start=True, stop=True)
                d["v2"] = v2

            def S7b(s):
                d = st[s]
                r2 = r2p.tile([120, NCOL], BF, tag="r2")
                nc.scalar.activation(r2[:], d.pop("v2")[:], AF.Abs_reciprocal_sqrt,
                                     bias=cs["epsv"][0:120, 0:1], scale=1.0 / D)
                d["r2"] = r2

            def S8(s):
                d = st[s]
                n2 = n2p.tile([120, NCOL], BF, tag="n2")
                nc.vector.tensor_mul(n2[:], d.pop("h2c")[:], d.pop("r2")[:])
                d["n2"] = n2

            def S9(s):
                d = st.pop(s)
                n2 = d["n2"]
                hp = hpp.tile([128, 480], F32, tag="hp")
                for c in range(4):
                    nc.tensor.matmul(
                        hp[:, 120 * c:120 * (c + 1)],
                        n2[:, 128 * c:128 * (c + 1)],
                        cs["whgblk"][:],
                        start=True, stop=True, skip_group_check=True)
                outf = outfp.tile([128, 480], F32, tag="outf")
                nc.vector.tensor_add(outf[:], hp[:], cs["bhnat"][:])
                nc.sync.dma_start(out=ov[s], in_=outf[:])

            load_quad(0)
            load_quad(1)
            GAP = lambda s: None
            stages = [S0, S1, GAP, S2a, S2b, S3, S4, S5, S6, GAP, S7a, S7b, S8, S9]
            for T in range(n_st + len(stages) - 1):
                for i, Si in enumerate(stages):
                    s = T - i
                    if 0 <= s < n_st:
                        Si(s)
    nc.compile()
    return nc


def _prep_x(x):
    B = x.shape[0]
    per = B // N_CORES
    unit = ROWS_ST * 4
    b_core = ((per + unit - 1) // unit) * unit
    n_quad = b_core // unit
    xb = np.asarray(x, dtype=ml_dtypes.bfloat16)
    shards = []
    for i in range(N_CORES):
        s = xb[i * per:(i + 1) * per]
        if b_core > per:
            s = np.concatenate([s, np.zeros((b_core - per, D_IN), ml_dtypes.bfloat16)])
        v = s.reshape(n_quad, 4, 128, 4, G, D_IN)          # g, st, b, c, t, f
        v = v.transpose(0, 4, 5, 1, 3, 2)                  # g, t, f, st, c, b
        shards.append(np.ascontiguousarray(v.reshape(n_quad, 100, 4 * NCOL)))
    return shards, per, b_core


LAST_EXEC_NS = None


def kernel(**inputs):
    x = np.asarray(inputs["x"], dtype=np.float32)
    consts = make_consts({k: np.asarray(v) for k, v in inputs.items() if k != "x"})
    shards, per, b_core = _prep_x(x)

    nc = build_nc(b_core)
    in_maps = []
    for i in range(N_CORES):
        m = {"x": shards[i]}
        for name, shape, ty in CONST_SPECS:
            m[name] = np.ascontiguousarray(
                consts[name].astype(ml_dtypes.bfloat16 if ty == "bf16" else np.float32))
        in_maps.append(m)

    from concourse import bass_utils
    res = bass_utils.run_bass_kernel_spmd(nc, in_maps, list(range(N_CORES)))
    global LAST_EXEC_NS
    LAST_EXEC_NS = res.exec_time_ns
    outs = [res.results[c]["out"].reshape(-1, D) for c in range(N_CORES)]
    return np.concatenate([r[:per] for r in outs], axis=0).astype(np.float32)


def _reference_np(x, w):
    x = x.astype(np.float64)
    f = {k: np.asarray(v, np.float64) for k, v in w.items()}

    def ln(h, g, b):
        mu = h.mean(-1, keepdims=True)
        var = ((h - mu) ** 2).mean(-1, keepdims=True)
        return (h - mu) / np.sqrt(var + EPS) * g + b

    h = x @ f["w_embed"] + f["b_embed"]
    n = ln(h, f["g_norm1"], f["b_norm1"])
    m = np.maximum(n @ f["w_fc1"] + f["b_fc1"], 0) @ f["w_fc2"] + f["b_fc2"]
    h = h + m
    h = ln(h, f["g_normh"], f["b_normh"])
    return h @ f["w_head"] + f["b_head"]


if __name__ == "__main__":
    from concourse.bass_interp import CoreSim
    import concourse.bass_interp as _bi
    import concourse.mybir as _mb

    _orig_act = _bi.InstructionExecutor.visit_InstActivation

    def _patched_act(self, instruction, **kw):
        if instruction.func == _mb.ActivationFunctionType.Abs_reciprocal_sqrt:
            instruction.func = _mb.ActivationFunctionType.Rsqrt
        return _orig_act(self, instruction, **kw)

    _bi.InstructionExecutor.visit_InstActivation = _patched_act

    rng = np.random.default_rng(0)
    b_core = ROWS_ST * 4
    w = {
        "w_embed": rng.uniform(-0.3, 0.3, (D_IN, D)).astype(np.float32),
        "b_embed": rng.uniform(-0.3, 0.3, (D,)).astype(np.float32),
        "g_norm1": np.ones(D, np.float32), "b_norm1": np.zeros(D, np.float32),
        "w_fc1": rng.uniform(-0.3, 0.3, (D, D)).astype(np.float32),
        "b_fc1": rng.uniform(-0.3, 0.3, (D,)).astype(np.float32),
        "w_fc2": rng.uniform(-0.3, 0.3, (D, D)).astype(np.float32),
        "b_fc2": rng.uniform(-0.3, 0.3, (D,)).astype(np.float32),
        "g_normh": np.ones(D, np.float32), "b_normh": np.zeros(D, np.float32),
        "w_head": rng.uniform(-0.3, 0.3, (D, D)).astype(np.float32),
        "b_head": rng.uniform(-0.3, 0.3, (D,)).astype(np.float32),
    }
    x = rng.standard_normal((b_core, D_IN)).astype(np.float32)
    consts = make_consts(w)

    xb = x.astype(ml_dtypes.bfloat16)
    v = xb.reshape(1, 4, 128, 4, G, D_IN).transpose(0, 4, 5, 1, 3, 2)
    xprep = np.ascontiguousarray(v.reshape(1, 100, 4 * NCOL))

    nc = build_nc(b_core)
    sim = CoreSim(nc, trace=False)
    sim.tensor("x")[:] = xprep
    for name, shape, ty in CONST_SPECS:
        sim.tensor(name)[:] = consts[name].astype(
            ml_dtypes.bfloat16 if ty == "bf16" else np.float32)
    sim.simulate(check_with_hw=False)
    got = np.asarray(sim.tensor("out"))

    ref = _reference_np(x, w)
    rel = np.linalg.norm(got - ref) / np.linalg.norm(ref)
    mx = np.abs(got - ref).max() / np.abs(ref).max()
    print(f"SIM rel_l2={rel:.3e}  scaled_absmax={mx:.3e}")
    assert rel < 2e-2, "simulation mismatch"
    print("SIM OK")
